# revision 10
# baseline (speedup 1.0000x reference)
"""Trainium2 Bass kernel for nn_Detector (region-sum pooling + softmax).

The reference computes softmax(x.reshape(B, H*W) @ filt) where filt is a
fixed 0/1 mask selecting 10 disjoint 113x113 rectangular regions of the
1024x1024 image.  The dense GEMM is really a sparse pooling: out[b, k]
is the sum of x[b] over region k.  Only ~12% of x is ever needed, so we
DMA exactly the 10 regions per image instead of streaming all 512 MB.

Distribution: data-parallel over batch, 8 NeuronCores x 16 images each.

All bulk loads ride the SWDGE (gpsimd) ring: a region row is one 452 B
descriptor and the ring is descriptor-rate-limited (~32 ns/desc across
16 SDMA engines) regardless of padding, alignment, or dtype, so exact
113-col windows minimize both descriptors and bytes.  Spreading regions
onto the HWDGE rings (sync/scalar) measures WORSE chip-wide (ring
mixing collapses all rings' packet rates), so everything stays on one
ring.  All 10 descriptor-generation instructions issue back-to-back on
Q7 (bufs=11, no tile-pool reuse stalls); each gen is ~1.3 us, well
ahead of the ~3.6 us/region drain.

Per region, one SWDGE DMA loads rows r0..r0+111 for all 16 images:
DRAM side is the plain monotonic 3D slice x[:, r0:r0+112, c0:c0+113];
SBUF side is [128, 14, 113] with partition = (batch, row-octet).  Both
sides enumerate elements in the same order, so no AP rearrange is
needed (SWDGE crashes on non-monotonic or 4D APs).  The last region is
split into two half-height DMAs (partition = (batch, 7-row group)) so
the final reduce only covers half a region after the last packet lands.
The 113th row of every region goes via small HWDGE DMAs on the sync
queue in parallel.

Compute: VectorE reduces each region tile -> [128, 1] partials in DMA
arrival order; one TensorE matmul with the 0/1 block indicator
[128, 16] contracts the 8 octets (or row-groups) per batch -> PSUM
[16, 11]; VectorE folds the split region's second half and adds the
remainder-row partials; ScalarE does the numerically-stable softmax.
"""

import numpy as np

import concourse.bass as bass
import concourse.tile as tile
from concourse import bacc, mybir
from concourse.bass_utils import run_bass_kernel_spmd

# Problem geometry — fixed by the reference's _build_filter(1024, 1024).
B, H, W = 128, 1024, 1024
S = 113  # min(1024 // 9, 1024 // 7)
REGIONS = [(2, 1), (2, 4), (2, 7), (4, 1), (4, 3), (4, 5), (4, 7), (6, 1), (6, 4), (6, 7)]
K = len(REGIONS)
N_CORES = 8
BPC = B // N_CORES  # images per core
F32 = mybir.dt.float32
OCT, GR = 8, 14  # 112 of the 113 region rows = 8 octets x 14 rows
HGR = 7          # row-group height for the split last region
K_SPLIT = K - 1


def host_blk():
    # blk[p, b] = 1 iff p // 8 == b: sums the 8 octets per batch.
    return np.repeat(np.eye(BPC, dtype=np.float32), OCT, axis=0)


def build_nc():
    nc = bacc.Bacc("TRN2", target_bir_lowering=False, debug=False)
    x = nc.declare_dram_parameter("x", [BPC, H, W], F32, isOutput=False)
    blk_d = nc.declare_dram_parameter("blk", [128, BPC], F32, isOutput=False)
    out = nc.declare_dram_parameter("out", [BPC, K], F32, isOutput=True)

    with tile.TileContext(nc) as tc:
        with (
            tc.tile_pool(name="reg", bufs=11) as rpool,
            tc.tile_pool(name="small", bufs=1) as spool,
            tc.tile_pool(name="psum", bufs=1, space=bass.MemorySpace.PSUM) as ppool,
        ):
            # All SWDGE region DMAs up front: Q7 generates descriptors
            # back-to-back while the SDMA engines drain behind it.
            tiles = []
            for k in range(K - 1):
                rb, cb = REGIONS[k]
                r0, c0 = rb * S, cb * S
                mt = rpool.tile([128, GR, S], F32, tag="mt")
                nc.gpsimd.dma_start(
                    out=mt[:], in_=x[:, r0:r0 + OCT * GR, c0:c0 + S]
                )
                tiles.append(mt)
            # Last region in two half-height DMAs: partition = (img,
            # 7-row group), so p // 8 is still the image and the halves
            # land in mpart columns 9 and 10.
            rb, cb = REGIONS[K_SPLIT]
            r9, c9 = rb * S, cb * S
            mt9a = rpool.tile([128, HGR, S], F32, tag="mt")
            nc.gpsimd.dma_start(
                out=mt9a[:], in_=x[:, r9:r9 + 8 * HGR, c9:c9 + S]
            )
            mt9b = rpool.tile([128, HGR, S], F32, tag="mt")
            nc.gpsimd.dma_start(
                out=mt9b[:], in_=x[:, r9 + 8 * HGR:r9 + 16 * HGR, c9:c9 + S]
            )

            # Remainder row (the 113th) of every region: tiny sync DMAs.
            rem = spool.tile([BPC, K, S], F32)
            for k, (rb, cb) in enumerate(REGIONS):
                r0, c0 = rb * S, cb * S
                nc.sync.dma_start(
                    out=rem[:, k, :], in_=x[:, r0 + OCT * GR, c0:c0 + S]
                )

            # Block indicator (host-provided — engine memsets can only
            # start at partition 0/32/64/96).
            blk = spool.tile([128, BPC], F32)
            nc.sync.dma_start(out=blk[:], in_=blk_d[:])

            # Per-region partials in DMA arrival order; columns 9 and 10
            # hold the split region's halves.
            mpart = spool.tile([128, K + 1], F32)
            for k in range(K - 1):
                nc.vector.reduce_sum(
                    out=mpart[:, k:k + 1], in_=tiles[k][:],
                    axis=mybir.AxisListType.XY,
                )
            rpart = spool.tile([BPC, K], F32)
            nc.vector.reduce_sum(out=rpart[:], in_=rem[:], axis=mybir.AxisListType.X)
            # The two halves of the last region reduce on different
            # engines in parallel: VectorE sums mt9a while ScalarE sums
            # mt9b (Copy activation with accumulate output).
            dummy = spool.tile([128, HGR, S], F32)
            nc.scalar.activation(
                dummy[:], mt9b[:], mybir.ActivationFunctionType.Copy,
                accum_out=mpart[:, 10:11],
            )
            nc.vector.reduce_sum(
                out=mpart[:, 9:10], in_=mt9a[:], axis=mybir.AxisListType.XY
            )

            # Contract the 8 octets (or row-groups) per batch.  Columns
            # 0:10 only need VectorE's partials, so that matmul issues
            # without waiting on ScalarE's mt9b accumulation.
            py = ppool.tile([BPC, K + 1], F32)
            nc.tensor.matmul(py[:, 0:K], blk[:], mpart[:, 0:K], start=True, stop=True)
            nc.tensor.matmul(
                py[:, K:K + 1], blk[:], mpart[:, K:K + 1], start=True, stop=True
            )

            ys = spool.tile([BPC, K], F32)
            nc.vector.tensor_add(ys[:], py[:, 0:K], rpart[:])
            nc.vector.tensor_add(ys[:, 9:10], ys[:, 9:10], py[:, 10:11])

            # Softmax over the 10 detectors, batches on partitions.
            m = spool.tile([BPC, 1], F32)
            nc.vector.reduce_max(m[:], ys[:], axis=mybir.AxisListType.X)
            negm = spool.tile([BPC, 1], F32)
            nc.vector.tensor_scalar_mul(negm[:], m[:], -1.0)
            e = spool.tile([BPC, K], F32)
            ssum = spool.tile([BPC, 1], F32)
            nc.scalar.activation(
                e[:], ys[:], mybir.ActivationFunctionType.Exp,
                bias=negm[:], accum_out=ssum[:],
            )
            rcp = spool.tile([BPC, 1], F32)
            nc.vector.reciprocal(rcp[:], ssum[:])
            o = spool.tile([BPC, K], F32)
            nc.scalar.mul(o[:], e[:], rcp[:])
            nc.sync.dma_start(out=out[:], in_=o[:])

    nc.compile()
    return nc


_NC = None


def get_nc():
    global _NC
    if _NC is None:
        _NC = build_nc()
    return _NC


def kernel(x, filt=None, **_unused):
    nc = get_nc()
    x = np.ascontiguousarray(np.asarray(x, dtype=np.float32))
    assert x.shape == (B, H, W), x.shape
    blk = host_blk()
    in_maps = [
        {"x": x[i * BPC:(i + 1) * BPC], "blk": blk} for i in range(N_CORES)
    ]
    res = run_bass_kernel_spmd(nc, in_maps, list(range(N_CORES)))
    return np.concatenate([r["out"] for r in res.results], axis=0)
